# revision 1
# baseline (speedup 1.0000x reference)
"""MultiHeadSINDyAttention TRN2 kernel.

Reference computation (N=4, L=2048, E=512, H=8, h=64, FORECAST=8, DT=1):
    qkv = query @ Wqkv + bqkv ; q,k,v split into 8 heads of 64
    attn = causal-softmax(q k^T / 8) v                    per (batch, head)
    A_h = Xi_h - Xi_h^T ; x_j = attn (I+A_h)^j, j=1..8    (SINDy rollout)
    out[b, j] = concat_h(x_{j,h}) @ Wo + bo               [4, 8, 2048, 512]

Key algebraic fold: the rollout + output projection collapse into
    out[b, j] = sum_h attn_{b,h} @ Wt[j,h] + bo,  Wt[j,h] = (I+A_h)^j Wo_h
so the 8 sequential SINDy steps become 8 precomputed [512, 512] weights
(tiny host-side compute) and the device kernel is three dense matmul
stages + one causal-softmax attention stage.

Sharding: 8 cores = (batch b in 0..3) x (forecast half g in 0..1).
Each core computes attention for all 8 heads of its batch (attention work
is duplicated x2 across the g-pair; it is the cheapest stage) and the
output projection for its 4 forecast steps. Outputs are disjoint slices
of the full [4, 8, 2048, 512] result — the gather is pure concatenation.

On-device layout (per core): everything is computed "transposed"
(channels on partitions, sequence on the free axis) so that softmax's
P @ v runs without any transposes:
    qkT[c, s]  = Wqkv^T query^T        (lhsT = Wqkv slices, rhs = query^T)
    S_T[k, q]  = k_h q_h^T             (lhsT = kT_h, rhs = qT_h, K=64)
    E = exp(S_T / 8)                   (ACT, staircase-causal subranges)
    attnT[d|1, q] = [v_h | 1]^T E      (K=128 k-tiles; row 64 = rowsum D)
    attnT_h /= D                       (recip + PE ones-outer broadcast)
    out[q, e]  = attnT^T Wt[j]         (lhsT = attnT, K=512 channels)
All matmuls run as float32r (measured numerically identical to fp32
matmul on this HW, 4x faster). Causality at 128-granularity: for the
k-tile crossing the diagonal at offset j*128, only q-columns >= j*128
are computed and a single [128,128] triangle mask handles the diagonal.
"""

import os
import sys

for _p in ("/opt/trn_rl_repo", "/root/.axon_site/_ro/trn_rl_repo"):
    if os.path.isdir(_p) and _p not in sys.path:
        sys.path.insert(0, _p)

import numpy as np

import concourse.bass as bass
import concourse.mybir as mybir
from concourse.tile import TileContext
from concourse.bass_utils import run_bass_kernel_spmd

F32 = mybir.dt.float32
F32R = mybir.dt.float32r
AF = mybir.ActivationFunctionType

N_B, L, E, H, EH, FC = 4, 2048, 512, 8, 64, 8
NCORES = 8
KT = E // 128        # 4 k-tiles of 128 over the embedding dim
MT = L // 128        # 16 tiles of 128 over the sequence
QB = L // 512        # 4 query blocks of 512
SCALE = 1.0 / np.sqrt(EH)


def legalize_waits(nc):
    """This toolchain's walrus accepts only ONE sync wait per instruction.
    Split extras onto preceding same-engine NoOps (one wait each)."""
    ctr = 0
    for fn in nc.m.functions:
        for blk in fn.blocks:
            out = []
            changed = False
            for inst in blk.instructions:
                si = inst.sync_info
                if si is not None and len(si.on_wait) > 1:
                    for w in si.on_wait[:-1]:
                        out.append(
                            mybir.InstNoOp(
                                name=f"I-xwait-{ctr}",
                                engine=inst.engine,
                                sync_info=mybir.SyncInfo(
                                    on_wait=[w], on_update=[]
                                ),
                            )
                        )
                        ctr += 1
                    inst.sync_info = mybir.SyncInfo(
                        on_wait=[si.on_wait[-1]], on_update=list(si.on_update)
                    )
                    changed = True
                out.append(inst)
            if changed:
                blk.instructions = out
    return ctr


def build_program(with_bias: bool, group: int = 2, sbufs: int = 2,
                  qk_copy_eng: str = "scalar", ebufs: int = 3,
                  stages: str = "bacd", fast_free: bool = False,
                  nbufs: tuple = (3, 4), obufs: int = 3,
                  dedup: bool = True):
    """group: how many non-crossing k-tiles share one psum tile + exp op.
    sbufs: bufs for that psum pool (group*sbufs banks <= 4).
    dedup: each core computes only its 4 heads' attention; attnT is
    AllGather'd within core pairs (ranks 2b, 2b+1)."""
    nc = bass.Bass(target_bir_lowering=False)

    HL = H // 2 if dedup else H          # local heads
    EL = HL * EH                          # local channel width (q, k or v)

    qT = nc.dram_tensor("qT", [E, L], F32R, kind="ExternalInput")
    wqk = nc.dram_tensor("wqk", [E, 2 * EL], F32R, kind="ExternalInput")
    wv = nc.dram_tensor("wv", [E, EL], F32R, kind="ExternalInput")
    wt = nc.dram_tensor("wt", [FC // 2, E, E], F32R, kind="ExternalInput")
    bqk = nc.dram_tensor("bqk", [1, 2 * EL], F32R, kind="ExternalInput")
    bv = nc.dram_tensor("bv", [1, EL], F32R, kind="ExternalInput")
    bo = nc.dram_tensor("bo", [1, E], F32R, kind="ExternalInput")
    onesr = nc.dram_tensor("onesr", [1, 512], F32R, kind="ExternalInput")
    onesf = nc.dram_tensor("onesf", [1, 64], F32R, kind="ExternalInput")
    vones = nc.dram_tensor("vones", [128, MT, HL, 1], F32R, kind="ExternalInput")
    trid = nc.dram_tensor("trid", [128, 256], F32R, kind="ExternalInput")
    out_d = nc.dram_tensor("out", [FC // 2, L, E], F32, kind="ExternalOutput")

    with TileContext(nc) as tc:
        with (
            tc.tile_pool(name="const", bufs=1) as cpool,
            tc.tile_pool(name="big", bufs=1) as big,
            tc.tile_pool(name="wk", bufs=2) as wkp,
            tc.tile_pool(name="qk", bufs=2) as qkp,
            tc.tile_pool(name="es", bufs=ebufs) as esp,
            tc.tile_pool(name="nrm", bufs=4) as nrm,
            tc.tile_pool(name="wts", bufs=2) as wtsp,
            tc.tile_pool(name="ost", bufs=3) as ostp,
            tc.tile_pool(name="psmm", bufs=2, space="PSUM") as psmm,
            tc.tile_pool(name="pss", bufs=sbufs, space="PSUM") as pss,
            tc.tile_pool(name="psa", bufs=2, space="PSUM") as psa,
            tc.tile_pool(name="dram", bufs=1, space="DRAM") as dramp,
        ):
            # ---- persistent loads -------------------------------------
            qTs = []
            for kt in range(KT):
                t = big.tile([128, L], F32R, tag=f"qt{kt}", name=f"qt{kt}")
                nc.sync.dma_start(out=t[:, :], in_=qT[kt * 128:(kt + 1) * 128, :])
                qTs.append(t)
            wvs = big.tile([128, KT, EL], F32R, tag="wvs")
            nc.sync.dma_start(
                out=wvs[:, :, :],
                in_=wv.rearrange("(kt p) n -> p kt n", p=128),
            )
            tri = big.tile([128, 256], F32R, tag="tri")
            nc.sync.dma_start(out=tri[:, :], in_=trid[:, :])
            if with_bias:
                bqk_s = cpool.tile([1, 2 * E], F32R, tag="bqk")
                nc.sync.dma_start(out=bqk_s[0:1, :], in_=bqk[:, :])
                bv_s = cpool.tile([1, E], F32R, tag="bv")
                nc.sync.dma_start(out=bv_s[0:1, :], in_=bv[:, :])
                bo_s = cpool.tile([1, E], F32R, tag="bo")
                nc.sync.dma_start(out=bo_s[0:1, :], in_=bo[:, :])
            ones_s = cpool.tile([1, 512], F32R, tag="ones")
            nc.sync.dma_start(out=ones_s[0:1, :], in_=onesr[:, :])
            onesf_s = cpool.tile([1, 64], F32R, tag="onesf")
            nc.sync.dma_start(out=onesf_s[0:1, :], in_=onesf[:, :])

            # v1: [128part, seq-tile, local head, 64 v-dims + ones col]
            v1 = big.tile([128, MT, HL, EH + 1], F32R, tag="v1")
            nc.sync.dma_start(out=v1[:, :, :, EH:EH + 1], in_=vones[:, :, :, :])

            # ---- stage B: v projection --------------------------------
            for mt in range(MT if "b" in stages else 0):
                pv = psmm.tile([128, 512], F32, tag="mm")
                for kt in range(KT):
                    nc.tensor.matmul(
                        pv[:, 0:EL],
                        qTs[kt][:, mt * 128:(mt + 1) * 128],
                        wvs[:, kt, :],
                        start=(kt == 0),
                        stop=(kt == KT - 1) and not with_bias,
                    )
                if with_bias:
                    nc.tensor.matmul(
                        pv[:, 0:EL], ones_s[0:1, 0:128], bv_s[0:1, :],
                        start=False, stop=True,
                    )
                # one strided copy scatters all local heads' v-slices
                nc.vector.tensor_copy(
                    v1[:, mt, :, 0:EH],
                    pv[:, 0:EL].rearrange("p (h d) -> p h d", h=HL),
                )

            # ---- stages A + C interleaved per head pair ---------------
            attnT = []
            if dedup:
                # local attnT tiles (one per local head pair) + gathered
                # [rank, seq] tiles fed by the pairwise AllGather
                for hp in range(2):
                    attnT.append(
                        big.tile([128, L], F32R, tag=f"attL{hp}",
                                 name=f"attL{hp}")
                    )
                attg, ccin, ccout = [], [], []
                for hp in range(2):
                    attg.append(
                        big.tile([128, 2, L], F32R, tag=f"attg{hp}",
                                 name=f"attg{hp}")
                    )
                    ccin.append(
                        dramp.tile([128, L], F32R, tag=f"ccin{hp}",
                                   name=f"ccin{hp}")
                    )
                    ccout.append(
                        dramp.tile([2, 128, L], F32R, tag=f"ccout{hp}",
                                   name=f"ccout{hp}")
                    )
            else:
                for ct in range(KT):
                    attnT.append(
                        big.tile([128, L], F32R, tag=f"att{ct}",
                                 name=f"att{ct}")
                    )

            for hp in range(HL // 2 if "a" in stages else 0):
                # A: project q and k channel tiles for heads 2hp, 2hp+1
                qk_dst = {}
                k_m = (2 + hp) if dedup else (KT + hp)
                for which, m in (("q", hp), ("k", k_m)):
                    wtile = wkp.tile(
                        [128, KT, 128], F32R, tag=f"w{which}", name=f"w{which}"
                    )
                    nc.sync.dma_start(
                        out=wtile[:, :, :],
                        in_=wqk[:, m * 128:(m + 1) * 128].rearrange(
                            "(kt p) m -> p kt m", p=128
                        ),
                    )
                    dst = qkp.tile(
                        [128, L], F32R, tag=f"qk{which}", name=f"qk{which}"
                    )
                    for nb in range(QB):
                        pa = psmm.tile([128, 512], F32, tag="mm")
                        for kt in range(KT):
                            nc.tensor.matmul(
                                pa[:, :],
                                wtile[:, kt, :],
                                qTs[kt][:, nb * 512:(nb + 1) * 512],
                                start=(kt == 0),
                                stop=(kt == KT - 1) and not with_bias,
                            )
                        if with_bias:
                            nc.tensor.matmul(
                                pa[:, :],
                                bqk_s[0:1, m * 128:(m + 1) * 128],
                                ones_s[0:1, :],
                                start=False, stop=True,
                            )
                        if qk_copy_eng == "scalar":
                            nc.scalar.copy(
                                dst[:, nb * 512:(nb + 1) * 512], pa[:, :]
                            )
                        else:
                            nc.vector.tensor_copy(
                                dst[:, nb * 512:(nb + 1) * 512], pa[:, :]
                            )
                    qk_dst[which] = dst

                # C: causal attention for the two heads, head-interleaved
                # (adjacent iterations are independent accumulation chains)
                for qb, hh in [(qb, hh) for qb in range(QB)
                               for hh in range(2 if "c" in stages else 0)]:
                    if True:
                        h = 2 * hp + hh
                        off = hh * EH
                        qrow = qk_dst["q"]
                        krow = qk_dst["k"]
                        pA = psa.tile([EH + 1, 512], F32, tag="attn")
                        q0 = qb * 512
                        # non-crossing k-tiles, exp'd `group` tiles at a time
                        for kt0 in range(0, 4 * qb, group):
                            g = min(group, 4 * qb - kt0)
                            # g S_T matmuls share the g-bank psum tile
                            pS = pss.tile([128, 512 * group], F32, tag="s")
                            for half in range(g):
                                kt = kt0 + half
                                nc.tensor.matmul(
                                    pS[:, half * 512:half * 512 + 512],
                                    krow[off:off + EH,
                                         kt * 128:kt * 128 + 128],
                                    qrow[off:off + EH, q0:q0 + 512],
                                    start=True, stop=True,
                                )
                            es = esp.tile([128, 512 * group], F32R, tag="es")
                            nc.scalar.activation(
                                es[:, 0:512 * g], pS[:, 0:512 * g], AF.Exp,
                                scale=float(SCALE),
                            )
                            for half in range(g):
                                nc.tensor.matmul(
                                    pA[:, :],
                                    v1[:, kt0 + half, h, :],
                                    es[:, half * 512:(half + 1) * 512],
                                    start=(kt0 + half == 0),
                                    stop=False,
                                )
                        # crossing k-tiles: only q-cols >= j*128 exist.
                        # Pack (j0,j1) and (j2,j3) into one psum tile each:
                        # one exp + one strided triangle-mul per pack.
                        for pk, (ja, jb) in enumerate(((0, 1), (2, 3))):
                            wa, wb = 512 - 128 * ja, 512 - 128 * jb
                            pS = pss.tile([128, 512 * group], F32, tag="s")
                            es = esp.tile([128, 1024], F32R, tag="esx", bufs=3)
                            for j, base in ((ja, 0), (jb, wa)):
                                kt = 4 * qb + j
                                w = 512 - 128 * j
                                nc.tensor.matmul(
                                    pS[:, base:base + w],
                                    krow[off:off + EH,
                                         kt * 128:kt * 128 + 128],
                                    qrow[off:off + EH, q0 + 128 * j:q0 + 512],
                                    start=True, stop=True,
                                )
                            nc.scalar.activation(
                                es[:, 0:wa + wb], pS[:, 0:wa + wb], AF.Exp,
                                scale=float(SCALE),
                            )
                            # both tiles' triangles sit at local cols 0 and wa
                            trv = es[:, 0:2 * wa].rearrange(
                                "p (j w) -> p j w", j=2
                            )[:, :, 0:128]
                            nc.vector.tensor_mul(
                                trv, trv,
                                tri[:, :].rearrange("p (j w) -> p j w", j=2),
                            )
                            for j, base in ((ja, 0), (jb, wa)):
                                kt = 4 * qb + j
                                w = 512 - 128 * j
                                nc.tensor.matmul(
                                    pA[:, 128 * j:512],
                                    v1[:, kt, h, :],
                                    es[:, base:base + w],
                                    start=(kt == 0),
                                    stop=(j == 3),
                                )
                        # normalize: attnT_h[:, qb] = pA[0:64] / D, D = pA[64].
                        if fast_free:
                            # Copy pA to SBUF so the PSUM bank frees after
                            # ONE op instead of the whole normalize chain.
                            sA = nrm.tile([EH + 1, 512], F32, tag="sA",
                                          bufs=nbufs[0])
                            nc.vector.tensor_copy(sA[:, :], pA[:, :])
                            num, dsrc = sA[0:EH, :], sA[EH:EH + 1, :]
                        else:
                            num, dsrc = pA[0:EH, :], pA[EH:EH + 1, :]
                        invd = nrm.tile([1, 512], F32R, tag="invd")
                        with nc.allow_low_precision(
                            reason="f32r is 32-bit storage; rounding only "
                            "at matmul consumption"
                        ):
                            nc.vector.reciprocal(invd[0:1, :], dsrc)
                        pB = psmm.tile([EH, 512], F32, tag="mm")
                        nc.tensor.matmul(
                            pB[:, :], onesf_s[0:1, :], invd[0:1, :],
                            start=True, stop=True,
                        )
                        sbb = nrm.tile([EH, 512], F32, tag="sbb",
                                       bufs=nbufs[1])
                        nc.vector.tensor_copy(sbb[:, :], pB[:, :])
                        nc.vector.tensor_mul(
                            attnT[h // 2][off:off + EH, q0:q0 + 512],
                            num,
                            sbb[:, :],
                        )

                if dedup and "c" in stages:
                    # pairwise AllGather of this head-pair's attnT
                    nc.gpsimd.dma_start(
                        out=ccin[hp][:, :], in_=attnT[hp][:, :]
                    )
                    nc.gpsimd.collective_compute(
                        "AllGather",
                        mybir.AluOpType.bypass,
                        replica_groups=[[0, 1], [2, 3], [4, 5], [6, 7]],
                        ins=[ccin[hp][:, :].opt()],
                        outs=[ccout[hp][:, :, :].opt()],
                    )
                    nc.gpsimd.dma_start(
                        out=attg[hp][:, :, :],
                        in_=ccout[hp].rearrange("r p n -> p r n"),
                    )

            # ---- stage D: output projection per forecast step ---------
            for n in range(FC // 2 if "d" in stages else 0):
                wts = wtsp.tile([128, KT, E], F32R, tag="wts")
                nc.sync.dma_start(
                    out=wts[:, :, :],
                    in_=wt[n].rearrange("(ct p) o -> p ct o", p=128),
                )
                for mt in range(MT):
                    pO = psmm.tile([128, 512], F32, tag="mm")
                    for ct in range(KT):
                        if dedup:
                            lhsT = attg[ct % 2][:, ct // 2,
                                               mt * 128:(mt + 1) * 128]
                        else:
                            lhsT = attnT[ct][:, mt * 128:(mt + 1) * 128]
                        nc.tensor.matmul(
                            pO[:, :],
                            lhsT,
                            wts[:, ct, :],
                            start=(ct == 0),
                            stop=(ct == KT - 1) and not with_bias,
                        )
                    if with_bias:
                        nc.tensor.matmul(
                            pO[:, :], ones_s[0:1, 0:128], bo_s[0:1, :],
                            start=False, stop=True,
                        )
                    ost = ostp.tile([128, 512], F32, tag="ost", bufs=obufs)
                    nc.vector.tensor_copy(ost[:, :], pO[:, :])
                    nc.sync.dma_start(
                        out=out_d[n, mt * 128:(mt + 1) * 128, :], in_=ost[:, :]
                    )

    legalize_waits(nc)
    return nc


_PROGRAMS = {}
DEDUP = True
BEST_KW = dict(obufs=5, ebufs=2, nbufs=(3, 2))


def _get_program(with_bias: bool):
    key = (with_bias, DEDUP)
    if key not in _PROGRAMS:
        _PROGRAMS[key] = build_program(with_bias, dedup=DEDUP, **BEST_KW)
    return _PROGRAMS[key]


def _host_inputs(query, Wqkv, bqkv, Wo, bo, Xi):
    """Per-core input maps. Core c = (batch c//2, forecast-half c%2)."""
    query = np.asarray(query, np.float32)
    Wqkv = np.asarray(Wqkv, np.float32)
    bqkv = np.asarray(bqkv, np.float32)
    Wo = np.asarray(Wo, np.float32)
    bo = np.asarray(bo, np.float32)
    Xi = np.asarray(Xi, np.float64)

    # Wt[j, h] = (I + Xi_h - Xi_h^T)^(j+1) @ Wo_h, stacked over h.
    A = Xi - np.swapaxes(Xi, -1, -2)
    B = np.eye(EH, dtype=np.float64)[None] + A          # [H, 64, 64]
    Wt = np.empty((FC, E, E), np.float32)
    Bp = np.broadcast_to(np.eye(EH, dtype=np.float64), (H, EH, EH)).copy()
    Wo64 = Wo.astype(np.float64).reshape(H, EH, E)
    for j in range(FC):
        Bp = Bp @ B
        Wt[j] = (Bp @ Wo64).reshape(E, E).astype(np.float32)

    kk = np.arange(128)[:, None]
    qq = np.arange(128)[None, :]
    tri1 = (qq >= kk).astype(np.float32)
    tri = np.concatenate([tri1, tri1], axis=1)  # [128, 256], two triangles

    onesr = np.ones((1, 512), np.float32)
    onesf = np.ones((1, 64), np.float32)
    bo_r = bo.reshape(1, -1)
    with_bias = bool(np.any(bqkv) or np.any(bo))

    EL = E // 2 if DEDUP else E
    vones = np.ones((128, MT, EL // EH, 1), np.float32)
    in_maps = []
    for c in range(NCORES):
        b, g = c // 2, c % 2
        if DEDUP:
            # this core owns heads 4g..4g+3: their q, k, v channel slices
            qs, ks, vs = (slice(g * EL, (g + 1) * EL),
                          slice(E + g * EL, E + (g + 1) * EL),
                          slice(2 * E + g * EL, 2 * E + (g + 1) * EL))
            wqk = np.ascontiguousarray(
                np.concatenate([Wqkv[:, qs], Wqkv[:, ks]], axis=1))
            wv = np.ascontiguousarray(Wqkv[:, vs])
            bqk = np.concatenate([bqkv[qs], bqkv[ks]]).reshape(1, -1)
            bv = np.ascontiguousarray(bqkv[vs]).reshape(1, -1)
        else:
            wqk = np.ascontiguousarray(Wqkv[:, : 2 * E])
            wv = np.ascontiguousarray(Wqkv[:, 2 * E:])
            bqk = np.ascontiguousarray(bqkv[: 2 * E]).reshape(1, -1)
            bv = np.ascontiguousarray(bqkv[2 * E:]).reshape(1, -1)
        in_maps.append({
            "qT": np.ascontiguousarray(query[b].T),
            "wqk": wqk,
            "wv": wv,
            "wt": np.ascontiguousarray(Wt[4 * g: 4 * g + 4]),
            "bqk": bqk,
            "bv": bv,
            "bo": bo_r,
            "onesr": onesr,
            "onesf": onesf,
            "vones": vones,
            "trid": tri,
        })
    return in_maps, with_bias


def _run(in_maps, with_bias, **kw):
    nc = _get_program(with_bias)
    return run_bass_kernel_spmd(nc, in_maps, list(range(NCORES)), **kw)


def kernel(query, key, value, Wqkv, bqkv, Wo, bo, Xi, _res_out=None, **kw):
    in_maps, with_bias = _host_inputs(query, Wqkv, bqkv, Wo, bo, Xi)
    res = _run(in_maps, with_bias, **kw)
    if _res_out is not None:
        _res_out.append(res)
    full = np.empty((N_B, FC, L, E), np.float32)
    for c in range(NCORES):
        b, g = c // 2, c % 2
        full[b, 4 * g: 4 * g + 4] = res.results[c]["out"]
    return full



# revision 6
# speedup vs baseline: 1.3095x; 1.3095x over previous
"""MultiHeadSINDyAttention TRN2 kernel (collective-free q-split sharding).

Reference computation (N=4, L=2048, E=512, H=8, h=64, FORECAST=8, DT=1):
    qkv = query @ Wqkv + bqkv ; q,k,v split into 8 heads of 64
    attn = causal-softmax(q k^T / 8) v                    per (batch, head)
    A_h = Xi_h - Xi_h^T ; x_j = attn (I+A_h)^j, j=1..8    (SINDy rollout)
    out[b, j] = concat_h(x_{j,h}) @ Wo + bo               [4, 8, 2048, 512]

Key algebraic fold: the rollout + output projection collapse into
    out[b, j] = sum_h attn_{b,h} @ Wt[j,h] + bo,  Wt[j,h] = (I+A_h)^j Wo_h
so the 8 sequential SINDy steps become 8 precomputed [512, 512] weights
(tiny host-side compute) and the device kernel is three dense matmul
stages + one causal-softmax attention stage.

Sharding: 8 cores = (batch b in 0..3) x (query-half g in 0..1) with NO
collectives.  Core (b, g) owns query tiles {4*qb + 2g, 4*qb + 2g + 1}
of every 512-wide block qb (1024 query positions total), and computes
  - k and v projections for the FULL sequence (duplicated within the
    b-pair; that is the price of losing the all-gather),
  - q projection + causal attention for ALL 8 heads on its owned
    1024 query positions (the causal area of the two owned-tile sets
    is identical, so load is balanced),
  - the output projection for all 8 forecast steps on its owned
    positions.
Outputs are disjoint: the host gather is pure indexing.

Causality with ONE instruction stream on every core: for each 512-wide
q block the kernel computes k-tiles 0..4qb-1 unmasked (always fully
valid for owned q positions) plus a fixed 4-k-tile "diagonal band"
(k-tiles 4qb..4qb+3) over the owned 256 columns.  Band causality is
applied as DATA: a per-core [128, 1024] tensor of {0, -1e30} is added
into the band PSUM with one identity-lhsT matmul per PSUM bank before
the exp, so masked entries exp to exactly 0 (and drop out of both the
numerator and the row-sum).  The mask pattern is qb-independent.

On-device layout per core: channels on partitions, sequence on the
free axis, so softmax's P @ v runs without transposes:
    S_T[k, q] = k_h q_h^T       (lhsT = kT_h, rhs = qT_h, K=64)
    E = exp(S_T / 8 + mask)     (ACT, groups of 4 k-tiles)
    attnT[d|1, q] = [v_h | 1]^T E   (row 64 = rowsum D)
    attnT_h /= D                (recip + PE ones-outer broadcast)
    out[q, e] = attnT^T Wt[j]   (lhsT = attnT, K=512 channels)
Matmuls run as float32r (same numerics as fp32 on this HW, 4x faster);
attnT, Wt and the output DMA use bf16 (well within tolerance) which
halves the tail-stage HBM traffic and keeps full PE rate.
"""

import os
import sys

for _p in ("/opt/trn_rl_repo", "/root/.axon_site/_ro/trn_rl_repo"):
    if os.path.isdir(_p) and _p not in sys.path:
        sys.path.insert(0, _p)

import numpy as np

import concourse.bass as bass
import concourse.mybir as mybir
from concourse.tile import TileContext
from concourse.bass_utils import run_bass_kernel_spmd

F32 = mybir.dt.float32
F32R = mybir.dt.float32r
BF16 = mybir.dt.bfloat16
AF = mybir.ActivationFunctionType

N_B, L, E, H, EH, FC = 4, 2048, 512, 8, 64, 8
NCORES = 8
KT = E // 128         # 4 channel tiles of 128
MT = L // 128         # 16 seq tiles of 128
LQ = L // 2           # 1024 owned query positions per core
QB = L // 512         # 4 query blocks of 512 (each owns 256 cols of each)
CW = 256              # owned columns per query block
SCALE = 1.0 / np.sqrt(EH)
NEG = -1.0e30


def legalize_waits(nc):
    """This toolchain's walrus accepts only ONE sync wait per instruction.
    Split extras onto preceding same-engine NoOps (one wait each)."""
    ctr = 0
    for fn in nc.m.functions:
        for blk in fn.blocks:
            out = []
            changed = False
            for inst in blk.instructions:
                si = inst.sync_info
                if si is not None and len(si.on_wait) > 1:
                    for w in si.on_wait[:-1]:
                        out.append(
                            mybir.InstNoOp(
                                name=f"I-xwait-{ctr}",
                                engine=inst.engine,
                                sync_info=mybir.SyncInfo(
                                    on_wait=[w], on_update=[]
                                ),
                            )
                        )
                        ctr += 1
                    inst.sync_info = mybir.SyncInfo(
                        on_wait=[si.on_wait[-1]], on_update=list(si.on_update)
                    )
                    changed = True
                out.append(inst)
            if changed:
                blk.instructions = out
    return ctr


def build_program(with_bias: bool, ebufs: int = 4, obufs: int = 4,
                  pssbufs: int = 2, use_sbb: bool = False):
    nc = bass.Bass(target_bir_lowering=False)

    qT = nc.dram_tensor("qT", [E, L], F32R, kind="ExternalInput")
    qTo = nc.dram_tensor("qTo", [E, LQ], F32R, kind="ExternalInput")
    wqk = nc.dram_tensor("wqk", [E, 2 * E], F32R, kind="ExternalInput")
    wv = nc.dram_tensor("wv", [E, E], F32R, kind="ExternalInput")
    wt = nc.dram_tensor("wt", [FC, E, E], BF16, kind="ExternalInput")
    maskR = nc.dram_tensor("maskR", [128, 4 * CW], F32R, kind="ExternalInput")
    eye = nc.dram_tensor("eye", [128, 128], F32R, kind="ExternalInput")
    onesf = nc.dram_tensor("onesf", [1, 64], F32R, kind="ExternalInput")
    vones = nc.dram_tensor("vones", [128, MT, H, 1], F32R, kind="ExternalInput")
    if with_bias:
        bqk = nc.dram_tensor("bqk", [1, 2 * E], F32R, kind="ExternalInput")
        bv = nc.dram_tensor("bv", [1, E], F32R, kind="ExternalInput")
        bo = nc.dram_tensor("bo", [1, E], F32R, kind="ExternalInput")
        onesr = nc.dram_tensor("onesr", [1, 512], F32R, kind="ExternalInput")
    out_d = nc.dram_tensor("out", [FC, LQ, E], BF16, kind="ExternalOutput")

    with TileContext(nc) as tc:
        with (
            tc.tile_pool(name="const", bufs=1) as cpool,
            tc.tile_pool(name="big", bufs=1) as big,
            tc.tile_pool(name="wk", bufs=2) as wkp,
            tc.tile_pool(name="qk", bufs=2) as qkp,
            tc.tile_pool(name="es", bufs=ebufs) as esp,
            tc.tile_pool(name="nrm", bufs=2) as nrm,
            tc.tile_pool(name="wts", bufs=2) as wtsp,
            tc.tile_pool(name="ost", bufs=obufs) as ostp,
            tc.tile_pool(name="psmm", bufs=2, space="PSUM") as psmm,
            tc.tile_pool(name="pss", bufs=pssbufs, space="PSUM") as pss,
            tc.tile_pool(name="psa", bufs=2, space="PSUM") as psa,
        ):
            # ---- persistent loads -------------------------------------
            qTs = big.tile([128, KT, L], F32R, tag="qts")
            nc.sync.dma_start(
                out=qTs[:, :, :], in_=qT.rearrange("(kt p) n -> p kt n", p=128)
            )
            qTos = big.tile([128, KT, LQ], F32R, tag="qtos")
            nc.sync.dma_start(
                out=qTos[:, :, :], in_=qTo.rearrange("(kt p) n -> p kt n", p=128)
            )
            wvs = big.tile([128, KT, E], F32R, tag="wvs")
            nc.sync.dma_start(
                out=wvs[:, :, :], in_=wv.rearrange("(kt p) n -> p kt n", p=128)
            )
            maskR_s = big.tile([128, 4 * CW], F32R, tag="mask")
            nc.sync.dma_start(out=maskR_s[:, :], in_=maskR[:, :])
            eye_s = cpool.tile([128, 128], F32R, tag="eye")
            nc.sync.dma_start(out=eye_s[:, :], in_=eye[:, :])
            onesf_s = cpool.tile([1, 64], F32R, tag="onesf")
            nc.sync.dma_start(out=onesf_s[0:1, :], in_=onesf[:, :])
            if with_bias:
                bqk_s = cpool.tile([1, 2 * E], F32R, tag="bqk")
                nc.sync.dma_start(out=bqk_s[0:1, :], in_=bqk[:, :])
                bv_s = cpool.tile([1, E], F32R, tag="bv")
                nc.sync.dma_start(out=bv_s[0:1, :], in_=bv[:, :])
                bo_s = cpool.tile([1, E], F32R, tag="bo")
                nc.sync.dma_start(out=bo_s[0:1, :], in_=bo[:, :])
                ones_s = cpool.tile([1, 512], F32R, tag="ones")
                nc.sync.dma_start(out=ones_s[0:1, :], in_=onesr[:, :])

            # v1: [128 seq-part, seq-tile, head, 64 v-dims + ones col]
            v1 = big.tile([128, MT, H, EH + 1], F32R, tag="v1")
            nc.sync.dma_start(out=v1[:, :, :, EH:EH + 1], in_=vones[:, :, :, :])

            # ---- stage B: v projection (full sequence) ----------------
            for mt in range(MT):
                pv = psmm.tile([128, 512], F32, tag="mm")
                for kt in range(KT):
                    nc.tensor.matmul(
                        pv[:, :],
                        qTs[:, kt, mt * 128:(mt + 1) * 128],
                        wvs[:, kt, :],
                        start=(kt == 0),
                        stop=(kt == KT - 1) and not with_bias,
                    )
                if with_bias:
                    nc.tensor.matmul(
                        pv[:, :], ones_s[0:1, 0:128], bv_s[0:1, :],
                        start=False, stop=True,
                    )
                nc.vector.tensor_copy(
                    v1[:, mt, :, 0:EH],
                    pv[:, :].rearrange("p (h d) -> p h d", h=H),
                )

            # attnT channel tiles: [128 chan (2 heads), owned 1024 q] bf16
            attnT = []
            for ct in range(KT):
                attnT.append(
                    big.tile([128, LQ], BF16, tag=f"att{ct}", name=f"att{ct}")
                )

            # ---- stages A + C interleaved per head pair ---------------
            for hp in range(H // 2):
                # A: project q (owned cols) and k (full seq) for heads
                # 2hp, 2hp+1 (one 128-channel slice of the weights).
                wt_q = wkp.tile([128, KT, 128], F32R, tag="wq", name="wq")
                nc.sync.dma_start(
                    out=wt_q[:, :, :],
                    in_=wqk[:, hp * 128:(hp + 1) * 128].rearrange(
                        "(kt p) m -> p kt m", p=128
                    ),
                )
                wt_k = wkp.tile([128, KT, 128], F32R, tag="wk", name="wk")
                nc.sync.dma_start(
                    out=wt_k[:, :, :],
                    in_=wqk[:, E + hp * 128:E + (hp + 1) * 128].rearrange(
                        "(kt p) m -> p kt m", p=128
                    ),
                )
                qrow = qkp.tile([128, LQ], F32R, tag="qrow", name="qrow")
                for nb in range(LQ // 512):
                    pa = psmm.tile([128, 512], F32, tag="mm")
                    for kt in range(KT):
                        nc.tensor.matmul(
                            pa[:, :],
                            wt_q[:, kt, :],
                            qTos[:, kt, nb * 512:(nb + 1) * 512],
                            start=(kt == 0),
                            stop=(kt == KT - 1) and not with_bias,
                        )
                    if with_bias:
                        nc.tensor.matmul(
                            pa[:, :],
                            bqk_s[0:1, hp * 128:(hp + 1) * 128],
                            ones_s[0:1, :],
                            start=False, stop=True,
                        )
                    nc.vector.tensor_copy(qrow[:, nb * 512:(nb + 1) * 512], pa)
                krow = qkp.tile([128, L], F32R, tag="krow", name="krow")
                for nb in range(L // 512):
                    pa = psmm.tile([128, 512], F32, tag="mm")
                    for kt in range(KT):
                        nc.tensor.matmul(
                            pa[:, :],
                            wt_k[:, kt, :],
                            qTs[:, kt, nb * 512:(nb + 1) * 512],
                            start=(kt == 0),
                            stop=(kt == KT - 1) and not with_bias,
                        )
                    if with_bias:
                        nc.tensor.matmul(
                            pa[:, :],
                            bqk_s[0:1, E + hp * 128:E + (hp + 1) * 128],
                            ones_s[0:1, :],
                            start=False, stop=True,
                        )
                    nc.vector.tensor_copy(krow[:, nb * 512:(nb + 1) * 512], pa)

                # C: causal attention for the two heads over owned q cols.
                # Per q block qb: groups of 4 k-tiles; group gi==qb is the
                # masked diagonal band.  Software-pipelined: PV trails the
                # S+exp of the next group so PE never waits on ACT.
                for qb, hh in [(qb, hh) for qb in range(QB)
                               for hh in range(2)]:
                    q0 = qb * CW
                    last_kt = 4 * qb + 3
                    # [65, 512] so the tile owns a full PSUM bank: two
                    # accumulation chains interleaved within one bank
                    # corrupt each other on HW.
                    pA = psa.tile([EH + 1, 2 * CW], F32, tag="attn")
                    es_t = {}

                    def emit_s(gi, hh):
                        off = hh * EH
                        band = gi == qb
                        pS = pss.tile([128, 4 * CW], F32, tag="s")
                        # mask first (start=True over each full bank), then
                        # scores accumulate on top: a start=False matmul only
                        # adds correctly onto a region initialized by a
                        # covering start=True write.
                        if band:
                            for half in range(2):
                                nc.tensor.matmul(
                                    pS[:, half * 512:(half + 1) * 512],
                                    eye_s[:, :],
                                    maskR_s[:, half * 512:(half + 1) * 512],
                                    start=True, stop=False,
                                )
                        for t in range(4):
                            kt = 4 * gi + t
                            nc.tensor.matmul(
                                pS[:, t * CW:(t + 1) * CW],
                                krow[off:off + EH, kt * 128:(kt + 1) * 128],
                                qrow[off:off + EH, q0:q0 + CW],
                                start=not band, stop=True,
                            )
                        es = esp.tile([128, 4 * CW], F32R, tag="es")
                        nc.scalar.activation(
                            es[:, :], pS[:, :], AF.Exp, scale=float(SCALE)
                        )
                        es_t[(gi, hh)] = es

                    def emit_pv(gi, hh):
                        h = 2 * hp + hh
                        es = es_t.pop((gi, hh))
                        for t in range(4):
                            kt = 4 * gi + t
                            nc.tensor.matmul(
                                pA[:, 0:CW],
                                v1[:, kt, h, :],
                                es[:, t * CW:(t + 1) * CW],
                                start=(kt == 0),
                                stop=(kt == last_kt),
                            )

                    for gi in range(qb + 1):
                        emit_s(gi, hh)
                        if gi > 0:
                            emit_pv(gi - 1, hh)
                    emit_pv(qb, hh)

                    # normalize: attnT = pA[0:64] * (1/rowsum)
                    invd = nrm.tile([1, CW], F32R, tag="invd")
                    with nc.allow_low_precision(
                        reason="f32r is 32-bit storage; rounding only at "
                        "matmul consumption"
                    ):
                        nc.vector.reciprocal(
                            invd[0:1, :], pA[EH:EH + 1, 0:CW]
                        )
                    pB = psmm.tile([128, 512], F32, tag="mm")
                    nc.tensor.matmul(
                        pB[0:EH, 0:CW], onesf_s[0:1, :], invd[0:1, :],
                        start=True, stop=True,
                    )
                    sbb = nrm.tile([EH, CW], F32, tag="sbb")
                    nc.vector.tensor_copy(sbb[:, :], pB[0:EH, 0:CW])
                    nc.vector.tensor_mul(
                        attnT[hp][hh * EH:(hh + 1) * EH, q0:q0 + CW],
                        pA[0:EH, 0:CW],
                        sbb[:, :],
                    )

            # ---- stage D: output projection per forecast step ---------
            for n in range(FC):
                wts = wtsp.tile([128, KT, E], BF16, tag="wts")
                nc.sync.dma_start(
                    out=wts[:, :, :],
                    in_=wt[n].rearrange("(ct p) o -> p ct o", p=128),
                )
                for mt in range(LQ // 128):
                    pO = psmm.tile([128, 512], F32, tag="mm")
                    for ct in range(KT):
                        nc.tensor.matmul(
                            pO[:, :],
                            attnT[ct][:, mt * 128:(mt + 1) * 128],
                            wts[:, ct, :],
                            start=(ct == 0),
                            stop=(ct == KT - 1) and not with_bias,
                        )
                    if with_bias:
                        nc.tensor.matmul(
                            pO[:, :], ones_s[0:1, 0:128], bo_s[0:1, :],
                            start=False, stop=True,
                        )
                    ost = ostp.tile([128, 512], BF16, tag="ost")
                    nc.scalar.copy(ost[:, :], pO[:, :])
                    nc.sync.dma_start(
                        out=out_d[n, mt * 128:(mt + 1) * 128, :], in_=ost[:, :]
                    )

    legalize_waits(nc)
    return nc


_PROGRAMS = {}
BEST_KW = dict(use_sbb=True)


def _get_program(with_bias: bool):
    key = (with_bias,)
    if key not in _PROGRAMS:
        _PROGRAMS[key] = build_program(with_bias, **BEST_KW)
    return _PROGRAMS[key]


def _host_inputs(query, Wqkv, bqkv, Wo, bo, Xi):
    """Per-core input maps. Core c = (batch c//2, query-half c%2)."""
    import ml_dtypes

    query = np.asarray(query, np.float32)
    Wqkv = np.asarray(Wqkv, np.float32)
    bqkv = np.asarray(bqkv, np.float32)
    Wo = np.asarray(Wo, np.float32)
    bo = np.asarray(bo, np.float32)
    Xi = np.asarray(Xi, np.float64)

    # Wt[j] = blockdiag_h (I + Xi_h - Xi_h^T)^(j+1) @ Wo, as [E, E].
    A = Xi - np.swapaxes(Xi, -1, -2)
    B = np.eye(EH, dtype=np.float64)[None] + A          # [H, 64, 64]
    Wt = np.empty((FC, E, E), np.float32)
    Bp = np.broadcast_to(np.eye(EH, dtype=np.float64), (H, EH, EH)).copy()
    Wo64 = Wo.astype(np.float64).reshape(H, EH, E)
    for j in range(FC):
        Bp = Bp @ B
        Wt[j] = (Bp @ Wo64).reshape(E, E).astype(np.float32)
    Wt_bf = Wt.astype(ml_dtypes.bfloat16)

    wqk = np.ascontiguousarray(Wqkv[:, :2 * E])
    wv = np.ascontiguousarray(Wqkv[:, 2 * E:])
    eye = np.eye(128, dtype=np.float32)
    onesf = np.ones((1, EH), np.float32)
    vones = np.ones((128, MT, H, 1), np.float32)
    onesr = np.ones((1, 512), np.float32)
    with_bias = bool(np.any(bqkv) or np.any(bo))

    # band mask, qb-independent: valid iff t*128 + kp <= g*256 + c
    kp = np.arange(128)[:, None]
    tc = np.arange(4 * CW)[None, :]
    t, c = tc // CW, tc % CW
    masks = []
    for g in range(2):
        valid = (t * 128 + kp) <= (g * CW + c)
        masks.append(np.where(valid, 0.0, NEG).astype(np.float32))

    in_maps = []
    for core in range(NCORES):
        b, g = core // 2, core % 2
        qT = np.ascontiguousarray(query[b].T)                # [E, L]
        qTo = np.ascontiguousarray(
            qT.reshape(E, QB, 2, CW)[:, :, g].reshape(E, LQ)
        )
        m = {
            "qT": qT,
            "qTo": qTo,
            "wqk": wqk,
            "wv": wv,
            "wt": Wt_bf,
            "maskR": masks[g],
            "eye": eye,
            "onesf": onesf,
            "vones": vones,
        }
        if with_bias:
            m["bqk"] = np.ascontiguousarray(bqkv[:2 * E]).reshape(1, -1)
            m["bv"] = np.ascontiguousarray(bqkv[2 * E:]).reshape(1, -1)
            m["bo"] = bo.reshape(1, -1)
            m["onesr"] = onesr
        in_maps.append(m)
    return in_maps, with_bias


def _run(in_maps, with_bias, **kw):
    nc = _get_program(with_bias)
    return run_bass_kernel_spmd(nc, in_maps, list(range(NCORES)), **kw)


def kernel(query, key, value, Wqkv, bqkv, Wo, bo, Xi, _res_out=None, **kw):
    in_maps, with_bias = _host_inputs(query, Wqkv, bqkv, Wo, bo, Xi)
    res = _run(in_maps, with_bias, **kw)
    if _res_out is not None:
        _res_out.append(res)
    full = np.empty((N_B, FC, L, E), np.float32)
    view = full.reshape(N_B, FC, QB, 2, CW, E)
    for c in range(NCORES):
        b, g = c // 2, c % 2
        o = np.asarray(res.results[c]["out"]).astype(np.float32)
        view[b, :, :, g] = o.reshape(FC, QB, CW, E)
    return full


# revision 22
# speedup vs baseline: 1.4168x; 1.0820x over previous
"""MultiHeadSINDyAttention TRN2 kernel (collective-free q-split sharding).

Reference computation (N=4, L=2048, E=512, H=8, h=64, FORECAST=8, DT=1):
    qkv = query @ Wqkv + bqkv ; q,k,v split into 8 heads of 64
    attn = causal-softmax(q k^T / 8) v                    per (batch, head)
    A_h = Xi_h - Xi_h^T ; x_j = attn (I+A_h)^j, j=1..8    (SINDy rollout)
    out[b, j] = concat_h(x_{j,h}) @ Wo + bo               [4, 8, 2048, 512]

Key algebraic fold: the rollout + output projection collapse into
    out[b, j] = sum_h attn_{b,h} @ Wt[j,h] + bo,  Wt[j,h] = (I+A_h)^j Wo_h
so the 8 sequential SINDy steps become 8 precomputed [512, 512] weights
(tiny host-side compute) and the device kernel is three dense matmul
stages + one causal-softmax attention stage.

Sharding: 8 cores = (batch b in 0..3) x (query-half g in 0..1) with NO
collectives.  Core (b, g) owns query tiles {4*qb + 2g, 4*qb + 2g + 1}
of every 512-wide block qb (1024 query positions total), and computes
  - k and v projections for the FULL sequence (duplicated within the
    b-pair; that is the price of losing the all-gather),
  - q projection + causal attention for ALL 8 heads on its owned
    1024 query positions (the causal area of the two owned-tile sets
    is identical, so load is balanced),
  - the output projection for all 8 forecast steps on its owned
    positions.
Outputs are disjoint: the host gather is pure indexing.

Causality with ONE instruction stream on every core: for each 512-wide
q block the kernel computes k-tiles 0..4qb-1 unmasked (always fully
valid for owned q positions) plus a fixed 4-k-tile "diagonal band"
(k-tiles 4qb..4qb+3) over the owned 256 columns.  Band causality is
applied as DATA: a per-core [128, 1024] tensor of {0, -1e30} is added
into the band PSUM with one identity-lhsT matmul per PSUM bank before
the exp, so masked entries exp to exactly 0 (and drop out of both the
numerator and the row-sum).  The mask pattern is qb-independent.

On-device layout per core: channels on partitions, sequence on the
free axis, so softmax's P @ v runs without transposes:
    S_T[k, q] = k_h q_h^T       (lhsT = kT_h, rhs = qT_h, K=64)
    E = exp(S_T / 8 + mask)     (ACT, groups of 4 k-tiles)
    attnT[d|1, q] = [v_h | 1]^T E   (row 64 = rowsum D)
    attnT_h /= D                (recip + PE ones-outer broadcast)
    out[q, e] = attnT^T Wt[j]   (lhsT = attnT, K=512 channels)
Matmuls run as float32r (same numerics as fp32 on this HW, 4x faster);
attnT, Wt and the output DMA use bf16 (well within tolerance) which
halves the tail-stage HBM traffic and keeps full PE rate.
"""

import os
import sys

for _p in ("/opt/trn_rl_repo", "/root/.axon_site/_ro/trn_rl_repo"):
    if os.path.isdir(_p) and _p not in sys.path:
        sys.path.insert(0, _p)

import numpy as np

import concourse.bass as bass
import concourse.mybir as mybir
from concourse.tile import TileContext
from concourse.bass_utils import run_bass_kernel_spmd

F32 = mybir.dt.float32
F32R = mybir.dt.float32r
BF16 = mybir.dt.bfloat16
AF = mybir.ActivationFunctionType

N_B, L, E, H, EH, FC = 4, 2048, 512, 8, 64, 8
NCORES = 8
KT = E // 128         # 4 channel tiles of 128
MT = L // 128         # 16 seq tiles of 128
LQ = L // 2           # 1024 owned query positions per core
QB = L // 512         # 4 query blocks of 512 (each owns 256 cols of each)
CW = 256              # owned columns per query block
SCALE = 1.0 / np.sqrt(EH)
NEG = -1.0e30


def legalize_waits(nc):
    """This toolchain's walrus accepts only ONE sync wait per instruction.
    Split extras onto preceding same-engine NoOps (one wait each)."""
    ctr = 0
    for fn in nc.m.functions:
        for blk in fn.blocks:
            out = []
            changed = False
            for inst in blk.instructions:
                si = inst.sync_info
                if si is not None and len(si.on_wait) > 1:
                    for w in si.on_wait[:-1]:
                        out.append(
                            mybir.InstNoOp(
                                name=f"I-xwait-{ctr}",
                                engine=inst.engine,
                                sync_info=mybir.SyncInfo(
                                    on_wait=[w], on_update=[]
                                ),
                            )
                        )
                        ctr += 1
                    inst.sync_info = mybir.SyncInfo(
                        on_wait=[si.on_wait[-1]], on_update=list(si.on_update)
                    )
                    changed = True
                out.append(inst)
            if changed:
                blk.instructions = out
    return ctr


def build_program(with_bias: bool, ebufs: int = 4, obufs: int = 4,
                  pssbufs: int = 2, wtsbufs: int = 3, use_sbb: bool = True):
    nc = bass.Bass(target_bir_lowering=False)

    qT = nc.dram_tensor("qT", [E, L], F32R, kind="ExternalInput")
    qTo = nc.dram_tensor("qTo", [E, LQ], F32R, kind="ExternalInput")
    wqk = nc.dram_tensor("wqk", [E, 2 * E], F32R, kind="ExternalInput")
    wv = nc.dram_tensor("wv", [E, E], F32R, kind="ExternalInput")
    wt = nc.dram_tensor("wt", [FC, E, E], BF16, kind="ExternalInput")
    maskR = nc.dram_tensor("maskR", [128, 4 * CW], F32R, kind="ExternalInput")
    eye = nc.dram_tensor("eye", [128, 128], F32R, kind="ExternalInput")
    onesf = nc.dram_tensor("onesf", [1, 64], F32R, kind="ExternalInput")
    if with_bias:
        bqk = nc.dram_tensor("bqk", [1, 2 * E], F32R, kind="ExternalInput")
        bv = nc.dram_tensor("bv", [1, E], F32R, kind="ExternalInput")
        bo = nc.dram_tensor("bo", [1, E], F32R, kind="ExternalInput")
        onesr = nc.dram_tensor("onesr", [1, 512], F32R, kind="ExternalInput")
    out_d = nc.dram_tensor("out", [FC, LQ, E], BF16, kind="ExternalOutput")

    with TileContext(nc) as tc:
        with (
            tc.tile_pool(name="const", bufs=1) as cpool,
            tc.tile_pool(name="big", bufs=1) as big,
            tc.tile_pool(name="wk", bufs=2) as wkp,
            tc.tile_pool(name="qk", bufs=2) as qkp,
            tc.tile_pool(name="es", bufs=ebufs) as esp,
            tc.tile_pool(name="nrm", bufs=2) as nrm,
            tc.tile_pool(name="wts", bufs=wtsbufs) as wtsp,
            tc.tile_pool(name="ost", bufs=obufs) as ostp,
            tc.tile_pool(name="psmm", bufs=2, space="PSUM") as psmm,
            tc.tile_pool(name="pss", bufs=pssbufs, space="PSUM") as pss,
            tc.tile_pool(name="psa", bufs=2, space="PSUM") as psa,
        ):
            # ---- per-head-pair A-stage state --------------------------
            wtq_t, wtk_t = {}, {}

            def emit_wt_dmas(hp):
                wt_q = wkp.tile([128, KT, 128], F32R, tag="wq", name="wq")
                nc.sync.dma_start(
                    out=wt_q[:, :, :],
                    in_=wqk[:, hp * 128:(hp + 1) * 128].rearrange(
                        "(kt p) m -> p kt m", p=128
                    ),
                )
                wt_k = wkp.tile([128, KT, 128], F32R, tag="wk", name="wk")
                nc.sync.dma_start(
                    out=wt_k[:, :, :],
                    in_=wqk[:, E + hp * 128:E + (hp + 1) * 128].rearrange(
                        "(kt p) m -> p kt m", p=128
                    ),
                )
                wtq_t[hp], wtk_t[hp] = wt_q, wt_k

            # ---- persistent loads -------------------------------------
            # DMA issue order is tuned so the FIFO feeds each consumer just
            # in time: B needs wvs + qT seq-chunks; the A/C tensors (wt
            # slices, qTo chunks, mask) arrive during B's compute.
            wvs = big.tile([128, KT, E], F32R, tag="wvs")
            nc.sync.dma_start(
                out=wvs[:, :, :], in_=wv.rearrange("(kt p) n -> p kt n", p=128)
            )
            qTs = big.tile([128, KT, L], F32R, tag="qts")
            qTos = big.tile([128, KT, LQ], F32R, tag="qtos")

            def load_qt(lo, hi):
                nc.sync.dma_start(
                    out=qTs[:, :, lo:hi],
                    in_=qT[:, lo:hi].rearrange("(kt p) n -> p kt n", p=128),
                )

            def load_qto(lo, hi):
                nc.sync.dma_start(
                    out=qTos[:, :, lo:hi],
                    in_=qTo[:, lo:hi].rearrange("(kt p) n -> p kt n", p=128),
                )

            maskR_s = big.tile([128, 4 * CW], F32R, tag="mask")
            eye_s = cpool.tile([128, 128], F32R, tag="eye")
            onesf_s = cpool.tile([1, 64], F32R, tag="onesf")

            load_qt(0, 256)
            load_qt(256, 512)
            emit_wt_dmas(0)
            load_qt(512, 1024)
            load_qt(1024, 1536)
            load_qto(0, 256)
            load_qt(1536, 2048)
            load_qto(256, 512)
            nc.sync.dma_start(out=maskR_s[:, :], in_=maskR[:, :])
            nc.sync.dma_start(out=eye_s[:, :], in_=eye[:, :])
            nc.sync.dma_start(out=onesf_s[0:1, :], in_=onesf[:, :])
            if with_bias:
                bqk_s = cpool.tile([1, 2 * E], F32R, tag="bqk")
                nc.sync.dma_start(out=bqk_s[0:1, :], in_=bqk[:, :])
                bv_s = cpool.tile([1, E], F32R, tag="bv")
                nc.sync.dma_start(out=bv_s[0:1, :], in_=bv[:, :])
                bo_s = cpool.tile([1, E], F32R, tag="bo")
                nc.sync.dma_start(out=bo_s[0:1, :], in_=bo[:, :])
                ones_s = cpool.tile([1, 512], F32R, tag="ones")
                nc.sync.dma_start(out=ones_s[0:1, :], in_=onesr[:, :])
            load_qto(512, 768)
            load_qto(768, 1024)

            # v1: [128 seq-part, seq-tile, head, 64 v-dims + ones col];
            # the ones column is memset on the idle gpsimd engine.
            v1 = big.tile([128, MT, H, EH + 1], F32R, tag="v1")
            # (f32r memset is not a legal ISA op; in0*0 + 1 on DVE is)
            nc.vector.tensor_scalar(
                v1[:, :, :, EH:EH + 1],
                eye_s[:, :].rearrange("p (a b c) -> p a b c", a=MT, b=H),
                0.0, 1.0, mybir.AluOpType.mult, mybir.AluOpType.add,
            )

            # ---- stage B: v projection (full sequence) ----------------
            for mt in range(MT):
                pv = psmm.tile([128, 512], F32, tag="mm")
                for kt in range(KT):
                    nc.tensor.matmul(
                        pv[:, :],
                        qTs[:, kt, mt * 128:(mt + 1) * 128],
                        wvs[:, kt, :],
                        start=(kt == 0),
                        stop=(kt == KT - 1) and not with_bias,
                    )
                if with_bias:
                    nc.tensor.matmul(
                        pv[:, :], ones_s[0:1, 0:128], bv_s[0:1, :],
                        start=False, stop=True,
                    )
                nc.vector.tensor_copy(
                    v1[:, mt, :, 0:EH],
                    pv[:, :].rearrange("p (h d) -> p h d", h=H),
                )

            # attnT channel tiles: [128 chan (2 heads), owned 1024 q] bf16
            attnT = []
            for ct in range(KT):
                attnT.append(
                    big.tile([128, LQ], BF16, tag=f"att{ct}", name=f"att{ct}")
                )

            # ---- stages A + C interleaved per head pair ---------------
            def make_astate(hp):
                """Allocates this head pair's qrow/krow tiles and returns
                (qproj, kproj) emit closures."""
                wt_q, wt_k = wtq_t[hp], wtk_t[hp]
                qrow = qkp.tile([128, LQ], F32R, tag="qrow", name="qrow")
                krow = qkp.tile([128, L], F32R, tag="krow", name="krow")

                def emit_qproj(nb):
                    pa = psmm.tile([128, 512], F32, tag="mm")
                    for kt in range(KT):
                        nc.tensor.matmul(
                            pa[:, :],
                            wt_q[:, kt, :],
                            qTos[:, kt, nb * 512:(nb + 1) * 512],
                            start=(kt == 0),
                            stop=(kt == KT - 1) and not with_bias,
                        )
                    if with_bias:
                        nc.tensor.matmul(
                            pa[:, :],
                            bqk_s[0:1, hp * 128:(hp + 1) * 128],
                            ones_s[0:1, :],
                            start=False, stop=True,
                        )
                    nc.vector.tensor_copy(qrow[:, nb * 512:(nb + 1) * 512], pa)

                def emit_kproj(nb):
                    pa = psmm.tile([128, 512], F32, tag="mm")
                    for kt in range(KT):
                        nc.tensor.matmul(
                            pa[:, :],
                            wt_k[:, kt, :],
                            qTs[:, kt, nb * 512:(nb + 1) * 512],
                            start=(kt == 0),
                            stop=(kt == KT - 1) and not with_bias,
                        )
                    if with_bias:
                        nc.tensor.matmul(
                            pa[:, :],
                            bqk_s[0:1, E + hp * 128:E + (hp + 1) * 128],
                            ones_s[0:1, :],
                            start=False, stop=True,
                        )
                    nc.vector.tensor_copy(krow[:, nb * 512:(nb + 1) * 512], pa)

                return qrow, krow, emit_qproj, emit_kproj

            def a_prologue(st):
                _, _, emit_qproj, emit_kproj = st
                emit_qproj(0)
                emit_kproj(0)
                emit_qproj(1)

            st = make_astate(0)
            a_prologue(st)

            for hp in range(H // 2):
                qrow, krow, emit_qproj, emit_kproj = st
                next_st = [None]

                # C: causal attention for the two heads over owned q cols.
                # Per q block qb: groups of 4 k-tiles; group gi==qb is the
                # masked diagonal band.  Software-pipelined: PV trails the
                # S+exp of the next group so PE never waits on ACT; the
                # next head pair's projections are injected before the last
                # combo's final PV to cover its exp latency.
                for ci, (qb, hh) in enumerate(
                    [(qb, hh) for qb in range(QB) for hh in range(2)]
                ):
                    if hh == 0 and qb > 0:
                        emit_kproj(qb)
                    if ci == 5 and hp + 1 < H // 2:
                        emit_wt_dmas(hp + 1)
                    tail_cb = None
                    if ci == 7 and hp + 1 < H // 2:
                        def tail_cb():
                            next_st[0] = make_astate(hp + 1)
                            a_prologue(next_st[0])
                    q0 = qb * CW
                    last_kt = 4 * qb + 3
                    # [65, 512] so the tile owns a full PSUM bank: two
                    # accumulation chains interleaved within one bank
                    # corrupt each other on HW.
                    pA = psa.tile([EH + 1, 2 * CW], F32, tag="attn")
                    es_t = {}

                    def emit_s(gi, hh):
                        off = hh * EH
                        band = gi == qb
                        pS = pss.tile([128, 4 * CW], F32, tag="s")
                        # mask first (start=True over each full bank), then
                        # scores accumulate on top: a start=False matmul only
                        # adds correctly onto a region initialized by a
                        # covering start=True write.
                        if band:
                            for half in range(2):
                                nc.tensor.matmul(
                                    pS[:, half * 512:(half + 1) * 512],
                                    eye_s[:, :],
                                    maskR_s[:, half * 512:(half + 1) * 512],
                                    start=True, stop=False,
                                )
                        for t in range(4):
                            kt = 4 * gi + t
                            nc.tensor.matmul(
                                pS[:, t * CW:(t + 1) * CW],
                                krow[off:off + EH, kt * 128:(kt + 1) * 128],
                                qrow[off:off + EH, q0:q0 + CW],
                                start=not band, stop=True,
                            )
                        es = esp.tile([128, 4 * CW], F32R, tag="es")
                        nc.scalar.activation(
                            es[:, :], pS[:, :], AF.Exp, scale=float(SCALE)
                        )
                        es_t[(gi, hh)] = es

                    def emit_pv(gi, hh):
                        h = 2 * hp + hh
                        es = es_t.pop((gi, hh))
                        for t in range(4):
                            kt = 4 * gi + t
                            nc.tensor.matmul(
                                pA[:, 0:CW],
                                v1[:, kt, h, :],
                                es[:, t * CW:(t + 1) * CW],
                                start=(kt == 0),
                                stop=(kt == last_kt),
                            )

                    for gi in range(qb + 1):
                        emit_s(gi, hh)
                        if gi > 0:
                            emit_pv(gi - 1, hh)
                    if tail_cb is not None:
                        tail_cb()
                    emit_pv(qb, hh)

                    # normalize: attnT = pA[0:64] * (1/rowsum)
                    invd = nrm.tile([1, CW], F32R, tag="invd")
                    with nc.allow_low_precision(
                        reason="f32r is 32-bit storage; rounding only at "
                        "matmul consumption"
                    ):
                        nc.vector.reciprocal(
                            invd[0:1, :], pA[EH:EH + 1, 0:CW]
                        )
                    pB = psmm.tile([128, 512], F32, tag="mm")
                    nc.tensor.matmul(
                        pB[0:EH, 0:CW], onesf_s[0:1, :], invd[0:1, :],
                        start=True, stop=True,
                    )
                    sbb = nrm.tile([EH, CW], F32, tag="sbb")
                    nc.vector.tensor_copy(sbb[:, :], pB[0:EH, 0:CW])
                    nc.vector.tensor_mul(
                        attnT[hp][hh * EH:(hh + 1) * EH, q0:q0 + CW],
                        pA[0:EH, 0:CW],
                        sbb[:, :],
                    )
                st = next_st[0]

            # ---- stage D: output projection per forecast step ---------
            wts_t = {}

            def load_wts(n):
                t = wtsp.tile([128, KT, E], BF16, tag="wts", name=f"wts{n}")
                nc.sync.dma_start(
                    out=t[:, :, :],
                    in_=wt[n].rearrange("(ct p) o -> p ct o", p=128),
                )
                wts_t[n] = t

            for n in range(min(wtsbufs, FC)):
                load_wts(n)
            for n in range(FC):
                wts = wts_t.pop(n)
                for mt in range(LQ // 128):
                    pO = psmm.tile([128, 512], F32, tag="mm")
                    for ct in range(KT):
                        nc.tensor.matmul(
                            pO[:, :],
                            attnT[ct][:, mt * 128:(mt + 1) * 128],
                            wts[:, ct, :],
                            start=(ct == 0),
                            stop=(ct == KT - 1) and not with_bias,
                        )
                    if with_bias:
                        nc.tensor.matmul(
                            pO[:, :], ones_s[0:1, 0:128], bo_s[0:1, :],
                            start=False, stop=True,
                        )
                    ost = ostp.tile([128, 512], BF16, tag="ost")
                    nc.scalar.copy(ost[:, :], pO[:, :])
                    nc.sync.dma_start(
                        out=out_d[n, mt * 128:(mt + 1) * 128, :], in_=ost[:, :]
                    )
                if n + wtsbufs < FC:
                    load_wts(n + wtsbufs)

    legalize_waits(nc)
    return nc


_PROGRAMS = {}
BEST_KW = dict(use_sbb=True)


def _get_program(with_bias: bool):
    key = (with_bias,)
    if key not in _PROGRAMS:
        _PROGRAMS[key] = build_program(with_bias, **BEST_KW)
    return _PROGRAMS[key]


def _host_inputs(query, Wqkv, bqkv, Wo, bo, Xi):
    """Per-core input maps. Core c = (batch c//2, query-half c%2)."""
    import ml_dtypes

    query = np.asarray(query, np.float32)
    Wqkv = np.asarray(Wqkv, np.float32)
    bqkv = np.asarray(bqkv, np.float32)
    Wo = np.asarray(Wo, np.float32)
    bo = np.asarray(bo, np.float32)
    Xi = np.asarray(Xi, np.float64)

    # Wt[j] = blockdiag_h (I + Xi_h - Xi_h^T)^(j+1) @ Wo, as [E, E].
    A = Xi - np.swapaxes(Xi, -1, -2)
    B = np.eye(EH, dtype=np.float64)[None] + A          # [H, 64, 64]
    Wt = np.empty((FC, E, E), np.float32)
    Bp = np.broadcast_to(np.eye(EH, dtype=np.float64), (H, EH, EH)).copy()
    Wo64 = Wo.astype(np.float64).reshape(H, EH, E)
    for j in range(FC):
        Bp = Bp @ B
        Wt[j] = (Bp @ Wo64).reshape(E, E).astype(np.float32)
    Wt_bf = Wt.astype(ml_dtypes.bfloat16)

    wqk = np.ascontiguousarray(Wqkv[:, :2 * E])
    wv = np.ascontiguousarray(Wqkv[:, 2 * E:])
    eye = np.eye(128, dtype=np.float32)
    onesf = np.ones((1, EH), np.float32)
    onesr = np.ones((1, 512), np.float32)
    with_bias = bool(np.any(bqkv) or np.any(bo))

    # band mask, qb-independent: valid iff t*128 + kp <= g*256 + c
    kp = np.arange(128)[:, None]
    tc = np.arange(4 * CW)[None, :]
    t, c = tc // CW, tc % CW
    masks = []
    for g in range(2):
        valid = (t * 128 + kp) <= (g * CW + c)
        masks.append(np.where(valid, 0.0, NEG).astype(np.float32))

    in_maps = []
    for core in range(NCORES):
        b, g = core // 2, core % 2
        qT = np.ascontiguousarray(query[b].T)                # [E, L]
        qTo = np.ascontiguousarray(
            qT.reshape(E, QB, 2, CW)[:, :, g].reshape(E, LQ)
        )
        m = {
            "qT": qT,
            "qTo": qTo,
            "wqk": wqk,
            "wv": wv,
            "wt": Wt_bf,
            "maskR": masks[g],
            "eye": eye,
            "onesf": onesf,
        }
        if with_bias:
            m["bqk"] = np.ascontiguousarray(bqkv[:2 * E]).reshape(1, -1)
            m["bv"] = np.ascontiguousarray(bqkv[2 * E:]).reshape(1, -1)
            m["bo"] = bo.reshape(1, -1)
            m["onesr"] = onesr
        in_maps.append(m)
    return in_maps, with_bias


def _run(in_maps, with_bias, **kw):
    nc = _get_program(with_bias)
    return run_bass_kernel_spmd(nc, in_maps, list(range(NCORES)), **kw)


def kernel(query, key, value, Wqkv, bqkv, Wo, bo, Xi, _res_out=None, **kw):
    in_maps, with_bias = _host_inputs(query, Wqkv, bqkv, Wo, bo, Xi)
    res = _run(in_maps, with_bias, **kw)
    if _res_out is not None:
        _res_out.append(res)
    full = np.empty((N_B, FC, L, E), np.float32)
    view = full.reshape(N_B, FC, QB, 2, CW, E)
    for c in range(NCORES):
        b, g = c // 2, c % 2
        o = np.asarray(res.results[c]["out"]).astype(np.float32)
        view[b, :, :, g] = o.reshape(FC, QB, CW, E)
    return full


# revision 24
# speedup vs baseline: 1.4612x; 1.0313x over previous
"""MultiHeadSINDyAttention TRN2 kernel (collective-free q-split sharding).

Reference computation (N=4, L=2048, E=512, H=8, h=64, FORECAST=8, DT=1):
    qkv = query @ Wqkv + bqkv ; q,k,v split into 8 heads of 64
    attn = causal-softmax(q k^T / 8) v                    per (batch, head)
    A_h = Xi_h - Xi_h^T ; x_j = attn (I+A_h)^j, j=1..8    (SINDy rollout)
    out[b, j] = concat_h(x_{j,h}) @ Wo + bo               [4, 8, 2048, 512]

Key algebraic fold: the rollout + output projection collapse into
    out[b, j] = sum_h attn_{b,h} @ Wt[j,h] + bo,  Wt[j,h] = (I+A_h)^j Wo_h
so the 8 sequential SINDy steps become 8 precomputed [512, 512] weights
(tiny host-side compute) and the device kernel is three dense matmul
stages + one causal-softmax attention stage.

Sharding: 8 cores = (batch b in 0..3) x (query-half g in 0..1) with NO
collectives.  Core (b, g) owns query tiles {4*qb + 2g, 4*qb + 2g + 1}
of every 512-wide block qb (1024 query positions total), and computes
  - k and v projections for the FULL sequence (duplicated within the
    b-pair; that is the price of losing the all-gather),
  - q projection + causal attention for ALL 8 heads on its owned
    1024 query positions (the causal area of the two owned-tile sets
    is identical, so load is balanced),
  - the output projection for all 8 forecast steps on its owned
    positions.
Outputs are disjoint: the host gather is pure indexing.

Causality with ONE instruction stream on every core: for each 512-wide
q block the kernel computes k-tiles 0..4qb-1 unmasked (always fully
valid for owned q positions) plus a fixed 4-k-tile "diagonal band"
(k-tiles 4qb..4qb+3) over the owned 256 columns.  Band causality is
applied as DATA: a per-core [128, 1024] tensor of {0, -1e30} is added
into the band PSUM with one identity-lhsT matmul per PSUM bank before
the exp, so masked entries exp to exactly 0 (and drop out of both the
numerator and the row-sum).  The mask pattern is qb-independent.

On-device layout per core: channels on partitions, sequence on the
free axis, so softmax's P @ v runs without transposes:
    S_T[k, q] = k_h q_h^T       (lhsT = kT_h, rhs = qT_h, K=64)
    E = exp(S_T / 8 + mask)     (ACT, groups of 4 k-tiles)
    attnT[d|1, q] = [v_h | 1]^T E   (row 64 = rowsum D)
    attnT_h /= D                (recip + PE ones-outer broadcast)
    out[q, e] = attnT^T Wt[j]   (lhsT = attnT, K=512 channels)
Matmuls run as float32r (same numerics as fp32 on this HW, 4x faster);
attnT, Wt and the output DMA use bf16 (well within tolerance) which
halves the tail-stage HBM traffic and keeps full PE rate.
"""

import os
import sys

for _p in ("/opt/trn_rl_repo", "/root/.axon_site/_ro/trn_rl_repo"):
    if os.path.isdir(_p) and _p not in sys.path:
        sys.path.insert(0, _p)

import numpy as np

import concourse.bass as bass
import concourse.mybir as mybir
from concourse.tile import TileContext
from concourse.bass_utils import run_bass_kernel_spmd

F32 = mybir.dt.float32
F32R = mybir.dt.float32r
BF16 = mybir.dt.bfloat16
AF = mybir.ActivationFunctionType

N_B, L, E, H, EH, FC = 4, 2048, 512, 8, 64, 8
NCORES = 8
KT = E // 128         # 4 channel tiles of 128
MT = L // 128         # 16 seq tiles of 128
LQ = L // 2           # 1024 owned query positions per core
QB = L // 512         # 4 query blocks of 512 (each owns 256 cols of each)
CW = 256              # owned columns per query block
SCALE = 1.0 / np.sqrt(EH)
NEG = -1.0e30


def legalize_waits(nc):
    """This toolchain's walrus accepts only ONE sync wait per instruction.
    Split extras onto preceding same-engine NoOps (one wait each)."""
    ctr = 0
    for fn in nc.m.functions:
        for blk in fn.blocks:
            out = []
            changed = False
            for inst in blk.instructions:
                si = inst.sync_info
                if si is not None and len(si.on_wait) > 1:
                    for w in si.on_wait[:-1]:
                        out.append(
                            mybir.InstNoOp(
                                name=f"I-xwait-{ctr}",
                                engine=inst.engine,
                                sync_info=mybir.SyncInfo(
                                    on_wait=[w], on_update=[]
                                ),
                            )
                        )
                        ctr += 1
                    inst.sync_info = mybir.SyncInfo(
                        on_wait=[si.on_wait[-1]], on_update=list(si.on_update)
                    )
                    changed = True
                out.append(inst)
            if changed:
                blk.instructions = out
    return ctr


def build_program(with_bias: bool, ebufs: int = 4, obufs: int = 4,
                  pssbufs: int = 2, wtsbufs: int = 3, use_sbb: bool = True):
    nc = bass.Bass(target_bir_lowering=False)

    qT = nc.dram_tensor("qT", [E, L], F32R, kind="ExternalInput")
    qTo = nc.dram_tensor("qTo", [E, LQ], F32R, kind="ExternalInput")
    wqk = nc.dram_tensor("wqk", [E, 2 * E], F32R, kind="ExternalInput")
    wv = nc.dram_tensor("wv", [E, E], F32R, kind="ExternalInput")
    wt = nc.dram_tensor("wt", [FC, E, E], BF16, kind="ExternalInput")
    maskR = nc.dram_tensor("maskR", [128, 4 * CW], F32R, kind="ExternalInput")
    eye = nc.dram_tensor("eye", [128, 128], F32R, kind="ExternalInput")
    onesf = nc.dram_tensor("onesf", [1, 64], F32R, kind="ExternalInput")
    if with_bias:
        bqk = nc.dram_tensor("bqk", [1, 2 * E], F32R, kind="ExternalInput")
        bv = nc.dram_tensor("bv", [1, E], F32R, kind="ExternalInput")
        bo = nc.dram_tensor("bo", [1, E], F32R, kind="ExternalInput")
        onesr = nc.dram_tensor("onesr", [1, 512], F32R, kind="ExternalInput")
    out_d = nc.dram_tensor("out", [FC, LQ, E], BF16, kind="ExternalOutput")

    with TileContext(nc) as tc:
        with (
            tc.tile_pool(name="const", bufs=1) as cpool,
            tc.tile_pool(name="big", bufs=1) as big,
            tc.tile_pool(name="wk", bufs=2) as wkp,
            tc.tile_pool(name="qk", bufs=2) as qkp,
            tc.tile_pool(name="es", bufs=ebufs) as esp,
            tc.tile_pool(name="nrm", bufs=2) as nrm,
            tc.tile_pool(name="wts", bufs=wtsbufs) as wtsp,
            tc.tile_pool(name="ost", bufs=obufs) as ostp,
            tc.tile_pool(name="psmm", bufs=2, space="PSUM") as psmm,
            tc.tile_pool(name="pss", bufs=pssbufs, space="PSUM") as pss,
            tc.tile_pool(name="psa", bufs=2, space="PSUM") as psa,
        ):
            # ---- per-head-pair A-stage state --------------------------
            wtq_t, wtk_t = {}, {}

            def emit_wt_dmas(hp):
                wt_q = wkp.tile([128, KT, 128], F32R, tag="wq", name="wq")
                nc.sync.dma_start(
                    out=wt_q[:, :, :],
                    in_=wqk[:, hp * 128:(hp + 1) * 128].rearrange(
                        "(kt p) m -> p kt m", p=128
                    ),
                )
                wt_k = wkp.tile([128, KT, 128], F32R, tag="wk", name="wk")
                nc.sync.dma_start(
                    out=wt_k[:, :, :],
                    in_=wqk[:, E + hp * 128:E + (hp + 1) * 128].rearrange(
                        "(kt p) m -> p kt m", p=128
                    ),
                )
                wtq_t[hp], wtk_t[hp] = wt_q, wt_k

            # ---- persistent loads -------------------------------------
            # DMA issue order is tuned so the FIFO feeds each consumer just
            # in time: B needs wvs + qT seq-chunks; the A/C tensors (wt
            # slices, qTo chunks, mask) arrive during B's compute.
            wvs = big.tile([128, KT, E], F32R, tag="wvs")
            nc.sync.dma_start(
                out=wvs[:, :, :], in_=wv.rearrange("(kt p) n -> p kt n", p=128)
            )
            qTs = big.tile([128, KT, L], F32R, tag="qts")
            qTos = big.tile([128, KT, LQ], F32R, tag="qtos")

            def load_qt(lo, hi):
                nc.sync.dma_start(
                    out=qTs[:, :, lo:hi],
                    in_=qT[:, lo:hi].rearrange("(kt p) n -> p kt n", p=128),
                )

            def load_qto(lo, hi):
                nc.sync.dma_start(
                    out=qTos[:, :, lo:hi],
                    in_=qTo[:, lo:hi].rearrange("(kt p) n -> p kt n", p=128),
                )

            maskR_s = big.tile([128, 4 * CW], F32R, tag="mask")
            eye_s = cpool.tile([128, 128], F32R, tag="eye")
            onesf_s = cpool.tile([1, 64], F32R, tag="onesf")

            load_qt(0, 256)
            load_qt(256, 512)
            emit_wt_dmas(0)
            load_qt(512, 1024)
            load_qt(1024, 1536)
            load_qto(0, 256)
            load_qt(1536, 2048)
            load_qto(256, 512)
            nc.sync.dma_start(out=maskR_s[:, :], in_=maskR[:, :])
            nc.sync.dma_start(out=eye_s[:, :], in_=eye[:, :])
            nc.sync.dma_start(out=onesf_s[0:1, :], in_=onesf[:, :])
            if with_bias:
                bqk_s = cpool.tile([1, 2 * E], F32R, tag="bqk")
                nc.sync.dma_start(out=bqk_s[0:1, :], in_=bqk[:, :])
                bv_s = cpool.tile([1, E], F32R, tag="bv")
                nc.sync.dma_start(out=bv_s[0:1, :], in_=bv[:, :])
                bo_s = cpool.tile([1, E], F32R, tag="bo")
                nc.sync.dma_start(out=bo_s[0:1, :], in_=bo[:, :])
                ones_s = cpool.tile([1, 512], F32R, tag="ones")
                nc.sync.dma_start(out=ones_s[0:1, :], in_=onesr[:, :])
            load_qto(512, 768)
            load_qto(768, 1024)

            # v1: [128 seq-part, seq-tile, head, 64 v-dims + ones col];
            # the ones column is memset on the idle gpsimd engine.
            v1 = big.tile([128, MT, H, EH + 1], F32R, tag="v1")
            # (f32r memset is not a legal ISA op; in0*0 + 1 on DVE is)
            nc.vector.tensor_scalar(
                v1[:, :, :, EH:EH + 1],
                eye_s[:, :].rearrange("p (a b c) -> p a b c", a=MT, b=H),
                0.0, 1.0, mybir.AluOpType.mult, mybir.AluOpType.add,
            )

            # ---- stage B: v projection (full sequence) ----------------
            for mt in range(MT):
                pv = psmm.tile([128, 512], F32, tag="mm")
                for kt in range(KT):
                    nc.tensor.matmul(
                        pv[:, :],
                        qTs[:, kt, mt * 128:(mt + 1) * 128],
                        wvs[:, kt, :],
                        start=(kt == 0),
                        stop=(kt == KT - 1) and not with_bias,
                    )
                if with_bias:
                    nc.tensor.matmul(
                        pv[:, :], ones_s[0:1, 0:128], bv_s[0:1, :],
                        start=False, stop=True,
                    )
                nc.vector.tensor_copy(
                    v1[:, mt, :, 0:EH],
                    pv[:, :].rearrange("p (h d) -> p h d", h=H),
                )

            # attnT channel tiles: [128 chan (2 heads), owned 1024 q] bf16
            attnT = []
            for ct in range(KT):
                attnT.append(
                    big.tile([128, LQ], BF16, tag=f"att{ct}", name=f"att{ct}")
                )

            # ---- stages A + C interleaved per head pair ---------------
            def make_astate(hp):
                """Allocates this head pair's qrow/krow tiles and returns
                (qproj, kproj) emit closures."""
                wt_q, wt_k = wtq_t[hp], wtk_t[hp]
                qrow = qkp.tile([128, LQ], F32R, tag="qrow", name="qrow")
                krow = qkp.tile([128, L], F32R, tag="krow", name="krow")

                def emit_qproj(nb):
                    pa = psmm.tile([128, 512], F32, tag="mm")
                    for kt in range(KT):
                        nc.tensor.matmul(
                            pa[:, :],
                            wt_q[:, kt, :],
                            qTos[:, kt, nb * 512:(nb + 1) * 512],
                            start=(kt == 0),
                            stop=(kt == KT - 1) and not with_bias,
                        )
                    if with_bias:
                        nc.tensor.matmul(
                            pa[:, :],
                            bqk_s[0:1, hp * 128:(hp + 1) * 128],
                            ones_s[0:1, :],
                            start=False, stop=True,
                        )
                    nc.vector.tensor_copy(qrow[:, nb * 512:(nb + 1) * 512], pa)

                def emit_kproj(nb):
                    pa = psmm.tile([128, 512], F32, tag="mm")
                    for kt in range(KT):
                        nc.tensor.matmul(
                            pa[:, :],
                            wt_k[:, kt, :],
                            qTs[:, kt, nb * 512:(nb + 1) * 512],
                            start=(kt == 0),
                            stop=(kt == KT - 1) and not with_bias,
                        )
                    if with_bias:
                        nc.tensor.matmul(
                            pa[:, :],
                            bqk_s[0:1, E + hp * 128:E + (hp + 1) * 128],
                            ones_s[0:1, :],
                            start=False, stop=True,
                        )
                    nc.vector.tensor_copy(krow[:, nb * 512:(nb + 1) * 512], pa)

                return qrow, krow, emit_qproj, emit_kproj

            def a_prologue(st):
                _, _, emit_qproj, emit_kproj = st
                emit_qproj(0)
                emit_kproj(0)
                emit_qproj(1)

            st = make_astate(0)
            a_prologue(st)

            # C: causal attention over owned q cols, software-pipelined at
            # two levels: within a combo, PV trails the S+exp of the next
            # group; across combos, the final (band) PV + normalize of combo
            # i are deferred until after combo i+1's first S-group, so the
            # band exp latency is always covered by PE work.  `pending`
            # holds the deferred tail of the previous combo.
            pending = [None]

            def flush_pending():
                if pending[0] is not None:
                    cb, pending[0] = pending[0], None
                    cb()

            for hp in range(H // 2):
                qrow, krow, emit_qproj, emit_kproj = st
                next_st = [None]

                for ci, (qb, hh) in enumerate(
                    [(qb, hh) for qb in range(QB) for hh in range(2)]
                ):
                    if hh == 0 and qb > 0:
                        emit_kproj(qb)
                    if ci == 5 and hp + 1 < H // 2:
                        emit_wt_dmas(hp + 1)
                    q0 = qb * CW
                    last_kt = 4 * qb + 3
                    # [65, 512] so the tile owns a full PSUM bank: two
                    # accumulation chains interleaved within one bank
                    # corrupt each other on HW.
                    pA = psa.tile([EH + 1, 2 * CW], F32, tag="attn")
                    es_t = {}

                    def emit_s(gi, qb=qb, hh=hh, q0=q0, es_t=es_t):
                        off = hh * EH
                        band = gi == qb
                        pS = pss.tile([128, 4 * CW], F32, tag="s")
                        # mask first (start=True over each full bank), then
                        # scores accumulate on top: a start=False matmul only
                        # adds correctly onto a region initialized by a
                        # covering start=True write.
                        if band:
                            for half in range(2):
                                nc.tensor.matmul(
                                    pS[:, half * 512:(half + 1) * 512],
                                    eye_s[:, :],
                                    maskR_s[:, half * 512:(half + 1) * 512],
                                    start=True, stop=False,
                                )
                        for t in range(4):
                            kt = 4 * gi + t
                            nc.tensor.matmul(
                                pS[:, t * CW:(t + 1) * CW],
                                krow[off:off + EH, kt * 128:(kt + 1) * 128],
                                qrow[off:off + EH, q0:q0 + CW],
                                start=not band, stop=True,
                            )
                        es = esp.tile([128, 4 * CW], F32R, tag="es")
                        nc.scalar.activation(
                            es[:, :], pS[:, :], AF.Exp, scale=float(SCALE)
                        )
                        es_t[gi] = es

                    def emit_pv(gi, hh=hh, pA=pA, es_t=es_t, last_kt=last_kt,
                                hp=hp):
                        h = 2 * hp + hh
                        es = es_t.pop(gi)
                        for t in range(4):
                            kt = 4 * gi + t
                            nc.tensor.matmul(
                                pA[:, 0:CW],
                                v1[:, kt, h, :],
                                es[:, t * CW:(t + 1) * CW],
                                start=(kt == 0),
                                stop=(kt == last_kt),
                            )

                    def combo_tail(qb=qb, hh=hh, q0=q0, pA=pA, hp=hp,
                                   emit_pv=emit_pv):
                        emit_pv(qb)
                        # normalize: attnT = pA[0:64] * (1/rowsum)
                        invd = nrm.tile([1, CW], F32R, tag="invd")
                        with nc.allow_low_precision(
                            reason="f32r is 32-bit storage; rounding only "
                            "at matmul consumption"
                        ):
                            nc.vector.reciprocal(
                                invd[0:1, :], pA[EH:EH + 1, 0:CW]
                            )
                        pB = psmm.tile([128, 512], F32, tag="mm")
                        nc.tensor.matmul(
                            pB[0:EH, 0:CW], onesf_s[0:1, :], invd[0:1, :],
                            start=True, stop=True,
                        )
                        sbb = nrm.tile([EH, CW], F32, tag="sbb")
                        nc.vector.tensor_copy(sbb[:, :], pB[0:EH, 0:CW])
                        nc.vector.tensor_mul(
                            attnT[hp][hh * EH:(hh + 1) * EH, q0:q0 + CW],
                            pA[0:EH, 0:CW],
                            sbb[:, :],
                        )

                    for gi in range(qb + 1):
                        emit_s(gi)
                        if gi == 0:
                            flush_pending()
                        if gi > 0:
                            emit_pv(gi - 1)
                    if ci == 7 and hp + 1 < H // 2:
                        next_st[0] = make_astate(hp + 1)
                        a_prologue(next_st[0])
                    pending[0] = combo_tail
                st = next_st[0]
            flush_pending()

            # ---- stage D: output projection per forecast step ---------
            wts_t = {}

            def load_wts(n):
                t = wtsp.tile([128, KT, E], BF16, tag="wts", name=f"wts{n}")
                nc.sync.dma_start(
                    out=t[:, :, :],
                    in_=wt[n].rearrange("(ct p) o -> p ct o", p=128),
                )
                wts_t[n] = t

            for n in range(min(wtsbufs, FC)):
                load_wts(n)
            for n in range(FC):
                wts = wts_t.pop(n)
                for mt in range(LQ // 128):
                    pO = psmm.tile([128, 512], F32, tag="mm")
                    for ct in range(KT):
                        nc.tensor.matmul(
                            pO[:, :],
                            attnT[ct][:, mt * 128:(mt + 1) * 128],
                            wts[:, ct, :],
                            start=(ct == 0),
                            stop=(ct == KT - 1) and not with_bias,
                        )
                    if with_bias:
                        nc.tensor.matmul(
                            pO[:, :], ones_s[0:1, 0:128], bo_s[0:1, :],
                            start=False, stop=True,
                        )
                    ost = ostp.tile([128, 512], BF16, tag="ost")
                    # alternate copy engines so the psum-bank release chain
                    # is never serialized behind a single engine's queue
                    if mt % 2 == 0:
                        nc.scalar.copy(ost[:, :], pO[:, :])
                    else:
                        nc.vector.tensor_copy(ost[:, :], pO[:, :])
                    nc.sync.dma_start(
                        out=out_d[n, mt * 128:(mt + 1) * 128, :], in_=ost[:, :]
                    )
                if n + wtsbufs < FC:
                    load_wts(n + wtsbufs)

    legalize_waits(nc)
    return nc


_PROGRAMS = {}
BEST_KW = dict(use_sbb=True)


def _get_program(with_bias: bool):
    key = (with_bias,)
    if key not in _PROGRAMS:
        _PROGRAMS[key] = build_program(with_bias, **BEST_KW)
    return _PROGRAMS[key]


def _host_inputs(query, Wqkv, bqkv, Wo, bo, Xi):
    """Per-core input maps. Core c = (batch c//2, query-half c%2)."""
    import ml_dtypes

    query = np.asarray(query, np.float32)
    Wqkv = np.asarray(Wqkv, np.float32)
    bqkv = np.asarray(bqkv, np.float32)
    Wo = np.asarray(Wo, np.float32)
    bo = np.asarray(bo, np.float32)
    Xi = np.asarray(Xi, np.float64)

    # Wt[j] = blockdiag_h (I + Xi_h - Xi_h^T)^(j+1) @ Wo, as [E, E].
    A = Xi - np.swapaxes(Xi, -1, -2)
    B = np.eye(EH, dtype=np.float64)[None] + A          # [H, 64, 64]
    Wt = np.empty((FC, E, E), np.float32)
    Bp = np.broadcast_to(np.eye(EH, dtype=np.float64), (H, EH, EH)).copy()
    Wo64 = Wo.astype(np.float64).reshape(H, EH, E)
    for j in range(FC):
        Bp = Bp @ B
        Wt[j] = (Bp @ Wo64).reshape(E, E).astype(np.float32)
    Wt_bf = Wt.astype(ml_dtypes.bfloat16)

    wqk = np.ascontiguousarray(Wqkv[:, :2 * E])
    wv = np.ascontiguousarray(Wqkv[:, 2 * E:])
    eye = np.eye(128, dtype=np.float32)
    onesf = np.ones((1, EH), np.float32)
    onesr = np.ones((1, 512), np.float32)
    with_bias = bool(np.any(bqkv) or np.any(bo))

    # band mask, qb-independent: valid iff t*128 + kp <= g*256 + c
    kp = np.arange(128)[:, None]
    tc = np.arange(4 * CW)[None, :]
    t, c = tc // CW, tc % CW
    masks = []
    for g in range(2):
        valid = (t * 128 + kp) <= (g * CW + c)
        masks.append(np.where(valid, 0.0, NEG).astype(np.float32))

    in_maps = []
    for core in range(NCORES):
        b, g = core // 2, core % 2
        qT = np.ascontiguousarray(query[b].T)                # [E, L]
        qTo = np.ascontiguousarray(
            qT.reshape(E, QB, 2, CW)[:, :, g].reshape(E, LQ)
        )
        m = {
            "qT": qT,
            "qTo": qTo,
            "wqk": wqk,
            "wv": wv,
            "wt": Wt_bf,
            "maskR": masks[g],
            "eye": eye,
            "onesf": onesf,
        }
        if with_bias:
            m["bqk"] = np.ascontiguousarray(bqkv[:2 * E]).reshape(1, -1)
            m["bv"] = np.ascontiguousarray(bqkv[2 * E:]).reshape(1, -1)
            m["bo"] = bo.reshape(1, -1)
            m["onesr"] = onesr
        in_maps.append(m)
    return in_maps, with_bias


def _run(in_maps, with_bias, **kw):
    nc = _get_program(with_bias)
    return run_bass_kernel_spmd(nc, in_maps, list(range(NCORES)), **kw)


def kernel(query, key, value, Wqkv, bqkv, Wo, bo, Xi, _res_out=None, **kw):
    in_maps, with_bias = _host_inputs(query, Wqkv, bqkv, Wo, bo, Xi)
    res = _run(in_maps, with_bias, **kw)
    if _res_out is not None:
        _res_out.append(res)
    full = np.empty((N_B, FC, L, E), np.float32)
    view = full.reshape(N_B, FC, QB, 2, CW, E)
    for c in range(NCORES):
        b, g = c // 2, c % 2
        o = np.asarray(res.results[c]["out"]).astype(np.float32)
        view[b, :, :, g] = o.reshape(FC, QB, CW, E)
    return full
